# revision 31
# baseline (speedup 1.0000x reference)
"""Distributed causal self-attention kernel for 8 TRN2 NeuronCores.

Problem (hardcoded): B=4, T=2048, C=1024, H=16 heads, D=64 head dim, fp32.
  y = softmax(causal(x Wq^T (x Wk^T)^T / sqrt(D))) (x Wv^T) Wp^T + biases

Sharding: data-parallel over B (4 groups) x tensor-parallel over heads
(2 groups of 8 heads).  Core c handles batch c//2, head-group c%2.  Each
core computes a partial output projection y_partial = O_g @ Wp[:,cols_g]^T;
the host sums the two partials of each batch pair (the 2-way all-reduce of
the sharding hint) and adds bp.

v3: one fused pipeline.  All matmul operands bf16 (host pre-casts; halves
DMA, 1 cycle/row on the PE).  The kernel is ACT(exp)-bound, so everything
else is arranged to hide under the exp stream and keep the PE busy enough
that the HAM clock gate stays at 2.4GHz:
  - QKV projection groups for t-blocks 1..3 are interleaved between the
    attention pair-rounds of earlier q-tiles (only block 0 runs up front);
    their PSUM accumulators share the broadcast/output-projection pool and
    their bias evictions run on the DVE, not ACT.
  - attention per (q-tile, head pair): S^T via two-head co-computation
    against zero-padded Q^T copies, exp on ACT (scale=1/8 fused, bf16 out),
    causal mask via gpsimd.affine_select, O^T accumulation with a ones
    column so PSUM row 64 is the softmax denominator.  O(kc-1) is emitted
    after S(kc) so the PE never sits behind exp(kc) in program order.
  - 1/denom = exp(-ln(denom)) on ACT (Ln+Exp share a table set; DVE
    InstReciprocal costs ~4us).  The denominator broadcast matmul and the
    final normalize multiply are DEFERRED into the next pair-round so the
    PSUM->SBUF->DMA->Ln->Exp chain latency hides behind real work.
  - the output projection y = O^T.T Wp^T for q-tile r is emitted one
    128-row tile per pair-round of q-tile r+1.
"""

import numpy as np
import ml_dtypes

import concourse.bass as bass
import concourse.mybir as mybir
from concourse.tile import TileContext
from concourse.bass_utils import run_bass_kernel_spmd

F32 = mybir.dt.float32
F32R = mybir.dt.float32r
BF16 = mybir.dt.bfloat16
AF = mybir.ActivationFunctionType
ALU = mybir.AluOpType

P = 128          # partitions
T = 2048         # sequence length
C = 1024         # model dim
D = 64           # head dim
HG = 8           # heads per core
J = HG * D       # per-core projection width (512)
CC = C // P      # contraction chunks over model dim (8)
JC = J // P      # j chunks (4)
NT = T // P      # 128-row t tiles (16)
TBS = 512        # t block size for phase-1 projections
NTB = T // TBS   # t blocks (4)
NQ = T // 512    # 512-wide query tiles (4)
NPAIR = HG // 2  # co-computed head pairs (4)

_CACHE = {}


def _split_excess_waits(nc):
    """Walrus in this container only accepts 1 sync-wait on CTRL-queue
    instructions (Drain etc.).  Hoist excess waits onto preceding nops on
    the same engine queue (program order makes this equivalent)."""
    n = 0
    for f in nc.m.functions:
        for bb in f.blocks:
            out = []
            for inst in bb.instructions:
                si = inst.sync_info
                limit = 1
                if si is not None and si.on_wait and len(si.on_wait) > limit:
                    waits = list(si.on_wait)
                    excess, keep = waits[:-limit], waits[-limit:]
                    for ci in range(0, len(excess), limit):
                        n += 1
                        out.append(mybir.InstNoOp(
                            name=f"waitsplit_{n}", opcode="nop", engine=inst.engine,
                            sync_info=mybir.SyncInfo(
                                on_wait=excess[ci:ci + limit], on_update=[]),
                        ))
                    inst.sync_info = mybir.SyncInfo(
                        on_wait=keep, on_update=list(si.on_update))
                out.append(inst)
            bb.instructions = out


def _build():
    nc = bass.Bass()
    # host passes x and the weight shards pre-transposed AND pre-cast to
    # bf16: xt = x[b].T, w?t = W?[rows].T, wpt = Wp[:,cols].T
    xt_in = nc.dram_tensor("xt", [C, T], BF16, kind="ExternalInput")
    wq_in = nc.dram_tensor("wqt", [C, J], BF16, kind="ExternalInput")
    wk_in = nc.dram_tensor("wkt", [C, J], BF16, kind="ExternalInput")
    wv_in = nc.dram_tensor("wvt", [C, J], BF16, kind="ExternalInput")
    wp_in = nc.dram_tensor("wpt", [J, C], BF16, kind="ExternalInput")
    bq_in = nc.dram_tensor("bq", [J], F32, kind="ExternalInput")
    bk_in = nc.dram_tensor("bk", [J], F32, kind="ExternalInput")
    bv_in = nc.dram_tensor("bv", [J], F32, kind="ExternalInput")
    sel2_in = nc.dram_tensor("sel2", [2, P], F32, kind="ExternalInput")
    y_out = nc.dram_tensor("y", [T, C], F32, kind="ExternalOutput")

    with TileContext(nc) as tc:
        with tc.tile_pool(name="persist", bufs=1) as persist:
            # persistent bf16 tensors
            qtz0 = persist.tile([P, JC, T], BF16, tag="qtz0")  # Q^T even heads
            qtz1 = persist.tile([P, JC, T], BF16, tag="qtz1")  # Q^T odd heads
            kt_t = persist.tile([P, JC, T], BF16, tag="kt")    # K^T
            v_t = persist.tile([P, NT, HG, D + 1], BF16, tag="v")  # V + ones
            ot_r = persist.tile([P, JC, T], BF16, tag="otr")   # normalized O^T
            wpt = persist.tile([P, JC, C], BF16, tag="wpt")    # Wp_g^T
            xt_all = persist.tile([P, CC, T], BF16, tag="xt")  # x^T
            wt_q = persist.tile([P, CC, J], BF16, tag="wtq")
            wt_k = persist.tile([P, CC, J], BF16, tag="wtk")
            wt_v = persist.tile([P, CC, J], BF16, tag="wtv")
            ones_row = persist.tile([1, P], F32R, tag="ones")
            bq_sb = persist.tile([P, JC], F32, tag="bq")
            bk_sb = persist.tile([P, JC], F32, tag="bk")
            bv_sb = persist.tile([1, J], F32, tag="bv")
            bv_r = persist.tile([1, J], F32R, tag="bvr")
            bv_bc = persist.tile([P, J], F32, tag="bvbc")
            ot_u = persist.tile([P, 512], BF16, tag="otu")
            tmp1 = persist.tile([D, 512], BF16, tag="tmp1")
            sel2 = persist.tile([2, P], F32R, tag="sel2")
            sel2f = persist.tile([2, P], F32, tag="sel2f")
            ones_f32 = persist.tile([P, P], F32, tag="ones_f32")
            ones_bf = persist.tile([P, P], BF16, tag="ones_bf")

            nc.gpsimd.memset(ones_f32[:], 1.0)
            nc.gpsimd.memset(ones_bf[:], 1.0)
            nc.vector.tensor_copy(ones_row[:], ones_f32[0:1, :])
            nc.gpsimd.memset(v_t[:, :, :, D:D + 1], 1.0)
            nc.gpsimd.memset(qtz0[D:P, :, :], 0.0)
            nc.gpsimd.memset(qtz1[0:D, :, :], 0.0)
            # small DMAs first (biases needed at the first evictions)
            nc.sync.dma_start(bq_sb[:], bq_in.rearrange("(o p) -> p o", p=P))
            nc.sync.dma_start(bk_sb[:], bk_in.rearrange("(o p) -> p o", p=P))
            nc.sync.dma_start(bv_sb[:], bv_in[None, :])
            nc.sync.dma_start(sel2f[:], sel2_in[:, :])
            nc.vector.tensor_copy(sel2[:], sel2f[:])
            # weight + x^T DMA queue: wq, x(tb0), wk, wv, x(tb1..3), wp
            for cc in range(CC):
                nc.sync.dma_start(wt_q[:, cc, :], wq_in[cc * P:(cc + 1) * P, :])
            for cc in range(CC):
                nc.sync.dma_start(xt_all[:, cc, 0:TBS],
                                  xt_in[cc * P:(cc + 1) * P, 0:TBS])
            for cc in range(CC):
                nc.sync.dma_start(wt_k[:, cc, :], wk_in[cc * P:(cc + 1) * P, :])
            for cc in range(CC):
                nc.sync.dma_start(wt_v[:, cc, :], wv_in[cc * P:(cc + 1) * P, :])
            for tb in range(1, NTB):
                ts = slice(tb * TBS, (tb + 1) * TBS)
                for cc in range(CC):
                    nc.sync.dma_start(xt_all[:, cc, ts],
                                      xt_in[cc * P:(cc + 1) * P, ts])
            for jc in range(JC):
                nc.sync.dma_start(wpt[:, jc, :], wp_in[jc * P:(jc + 1) * P, :])
            wts = {"q": wt_q, "k": wt_k, "v": wt_v}

            with (
                tc.tile_pool(name="e", bufs=4) as e_pool,
                tc.tile_pool(name="rc", bufs=2) as rc_pool,
                tc.tile_pool(name="yout", bufs=2) as y_pool,
                tc.tile_pool(name="ps_s", bufs=2, space="PSUM") as ps_s,
                tc.tile_pool(name="ps_o", bufs=2, space="PSUM") as ps_o,
                tc.tile_pool(name="ps_bc", bufs=2, space="PSUM") as ps_bc,
            ):
                # bv broadcast to all 128 partitions via K=1 matmul
                nc.vector.tensor_copy(bv_r[:], bv_sb[:])
                ps_bv = ps_bc.tile([P, 512], F32, tag="bc", name="ps_bv")
                nc.tensor.matmul(ps_bv[:], lhsT=ones_row[:], rhs=bv_r[:],
                                 start=True, stop=True)
                nc.vector.tensor_copy(bv_bc[:], ps_bv[:])

                # HAM warm-up: ~3.5us of full-array matmuls
                ps_warm = ps_s.tile([P, 1024], F32, tag="s", name="ps_warm")
                for _ in range(32):
                    nc.tensor.matmul(ps_warm[:, 0:P], lhsT=ones_bf[:],
                                     rhs=ones_bf[:], start=True, stop=True)

                # ---- QKV projection groups (bias evictions on DVE) --------
                # QKV groups are split into two ~0.85us micro-thunks (4
                # accumulating matmuls each) so they can be paced between
                # attention chunks without ever delaying the next S matmul
                # by more than the ACT slack.
                def qk_group(tb, name, jc):
                    ts = slice(tb * TBS, (tb + 1) * TBS)
                    wt, bias = wts[name], (bq_sb if name == "q" else bk_sb)
                    state = {}

                    def first():
                        psq = ps_bc.tile([P, 512], F32, tag="bc",
                                         name=f"psq_{tb}_{name}_{jc}")
                        state["psq"] = psq
                        for cc in range(4):
                            nc.tensor.matmul(
                                psq[:], lhsT=wt[:, cc, jc * P:(jc + 1) * P],
                                rhs=xt_all[:, cc, ts],
                                start=(cc == 0), stop=False)

                    def second():
                        psq = state["psq"]
                        for cc in range(4, CC):
                            nc.tensor.matmul(
                                psq[:], lhsT=wt[:, cc, jc * P:(jc + 1) * P],
                                rhs=xt_all[:, cc, ts],
                                start=False, stop=(cc == CC - 1))
                        if name == "q":
                            nc.vector.tensor_scalar_add(
                                qtz0[0:D, jc, ts], psq[0:D, :],
                                bias[0:D, jc:jc + 1])
                            nc.vector.tensor_scalar_add(
                                qtz1[D:P, jc, ts], psq[D:P, :],
                                bias[D:P, jc:jc + 1])
                        else:
                            nc.vector.tensor_scalar_add(
                                kt_t[:, jc, ts], psq[:], bias[:, jc:jc + 1])
                    return [first, second]

                def v_group(tb, sub):
                    tt = tb * (TBS // P) + sub
                    state = {}

                    def first():
                        psv = ps_bc.tile([P, 512], F32, tag="bc",
                                         name=f"psv_{tb}_{sub}")
                        state["psv"] = psv
                        for cc in range(4):
                            nc.tensor.matmul(
                                psv[:],
                                lhsT=xt_all[:, cc, tt * P:(tt + 1) * P],
                                rhs=wts["v"][:, cc, :],
                                start=(cc == 0), stop=False)

                    def second():
                        psv = state["psv"]
                        for cc in range(4, CC):
                            nc.tensor.matmul(
                                psv[:],
                                lhsT=xt_all[:, cc, tt * P:(tt + 1) * P],
                                rhs=wts["v"][:, cc, :],
                                start=False, stop=(cc == CC - 1))
                        nc.vector.tensor_tensor(
                            v_t[:, tt, :, 0:D],
                            psv.rearrange("p (h d) -> p h d", h=HG),
                            bv_bc.rearrange("p (h d) -> p h d", h=HG),
                            ALU.add)
                    return [first, second]

                def phase1_thunks(tb):
                    gs = []
                    for jc in range(JC):
                        gs.extend(qk_group(tb, "k", jc))
                    for jc in range(JC):
                        gs.extend(qk_group(tb, "q", jc))
                    for sub in range(TBS // P):
                        gs.extend(v_group(tb, sub))
                    return gs

                # t block 0 runs up front; blocks 1..3 are paced into the
                # attention rounds of q-tiles 0..2
                for g in phase1_thunks(0):
                    g()

                # ---- output projection, one 512-col half per thunk --------
                def phase3_thunks(tt):
                    ts = slice(tt * P, (tt + 1) * P)
                    state = {}

                    def half(nh):
                        if nh == 0:
                            state["y"] = y_pool.tile([P, C], F32, tag="y",
                                                     name=f"y_{tt}")
                        psy = ps_bc.tile([P, 512], F32, tag="bc",
                                         name=f"psy_{tt}_{nh}")
                        for jc in range(JC):
                            nc.tensor.matmul(
                                psy[:],
                                lhsT=ot_r[:, jc, ts],
                                rhs=wpt[:, jc, nh * 512:(nh + 1) * 512],
                                start=(jc == 0), stop=(jc == JC - 1))
                        nc.vector.tensor_copy(
                            state["y"][:, nh * 512:(nh + 1) * 512], psy[:])
                        if nh == 1:
                            nc.sync.dma_start(y_out[ts, :], state["y"][:])
                    return [lambda: half(0), lambda: half(1)]

                # ---- attention ------------------------------------------
                pending_recip = []
                pending_norm = []

                for qt in range(NQ):
                    nk = (qt + 1) * 4
                    qs = slice(qt * 512, (qt + 1) * 512)
                    # background PE work paced into this q-tile's rounds:
                    # next t-block's projections + previous q-tile's output
                    # projection, ~0.85us per thunk, one per chunk slot
                    # the previous q-tile's last normalize is still deferred;
                    # it must be emitted before any phase3 thunk that reads
                    # ot_r (emission order defines dependencies)
                    while pending_recip:
                        pending_recip.pop(0)()
                    while pending_norm:
                        pending_norm.pop(0)()
                    bg = []
                    if qt < NQ - 1:
                        bg.extend(phase1_thunks(qt + 1))
                    if qt > 0:
                        for i in range(4):
                            bg.extend(phase3_thunks((qt - 1) * 4 + i))
                    slots = NPAIR * nk
                    for pair in range(NPAIR):
                        pso = [ps_o.tile([P, 512], F32, tag="o",
                                         name=f"pso_{pair}_{qt}_{i}")
                               for i in range(2)]

                        def emit_o(kc, e, delta, pair=pair, nk=nk, pso=pso):
                            for half in range(2):
                                h = pair * 2 + half
                                nc.tensor.matmul(
                                    pso[half][0:D + 1, delta:],
                                    lhsT=v_t[:, kc, h, :],
                                    rhs=e[:, half * 512 + delta:
                                          (half + 1) * 512],
                                    start=(kc == 0), stop=(kc == nk - 1))

                        # software-pipelined: O(kc-1) is emitted after S(kc)
                        # so the PE never waits on exp(kc) in program order
                        prev = None
                        for kc in range(nk):
                            ks = slice(kc * P, (kc + 1) * P)
                            delta = max(0, (kc - qt * 4) * P)
                            # first two diagonal chunks: compute S full
                            # width (cheaper than splitting the exp)
                            sd = delta if delta >= 256 else 0
                            pss = ps_s.tile([P, 1024], F32, tag="s")
                            for half in range(2):
                                qtz = qtz0 if half == 0 else qtz1
                                nc.tensor.matmul(
                                    pss[:, half * 512 + sd:(half + 1) * 512],
                                    lhsT=kt_t[:, pair, ks],
                                    rhs=qtz[:, pair,
                                            qt * 512 + sd:(qt + 1) * 512],
                                    start=True, stop=True)
                            if prev is not None:
                                emit_o(*prev)
                            if kc == 1 and pending_recip:
                                pending_recip.pop(0)()
                            if kc == 3 and pending_norm:
                                pending_norm.pop(0)()
                            # paced background PE work (projections for the
                            # next t-block / previous q-tile's y matmuls)
                            n_bg = -(-len(bg) // slots) if bg else 0
                            for _ in range(min(n_bg, len(bg))):
                                bg.pop(0)()
                            slots -= 1
                            e = e_pool.tile([P, 1024], BF16, tag="e")
                            if sd == 0:
                                nc.scalar.activation(
                                    e[:], pss[:], AF.Exp, scale=0.125)
                            else:
                                for half in range(2):
                                    sl = slice(half * 512 + sd,
                                               (half + 1) * 512)
                                    nc.scalar.activation(
                                        e[:, sl], pss[:, sl], AF.Exp,
                                        scale=0.125)
                            if kc >= qt * 4:  # diagonal: causal mask
                                nc.gpsimd.affine_select(
                                    out=e[:], in_=e[:],
                                    compare_op=ALU.is_ge, fill=0.0,
                                    base=-delta, channel_multiplier=-1,
                                    pattern=[[0, 2], [1, 512]])
                            prev = (kc, e, delta)
                        emit_o(*prev)

                        # eviction: O rows to SBUF (odd half crosses
                        # partitions via DMA), denominators to partitions
                        # 0/1, 1/den = exp(-ln(den)) on ACT.
                        nc.vector.tensor_copy(ot_u[0:D, :], pso[0][0:D, :])
                        nc.vector.tensor_copy(tmp1[:], pso[1][0:D, :])
                        nc.sync.dma_start(ot_u[D:P, :], tmp1[:])
                        den = rc_pool.tile([2, 512], F32, tag="den",
                                           name=f"den_{pair}_{qt}")
                        lnd = rc_pool.tile([2, 512], F32, tag="lnd",
                                           name=f"lnd_{pair}_{qt}")
                        rr2 = rc_pool.tile([2, 512], F32R, tag="rr2",
                                           name=f"rr_{pair}_{qt}")
                        dsb0 = rc_pool.tile([D + 1, 512], F32, tag="dsb0",
                                            name=f"dsb0_{pair}_{qt}")
                        dsb1 = rc_pool.tile([D + 1, 512], F32, tag="dsb1",
                                            name=f"dsb1_{pair}_{qt}")
                        nc.vector.tensor_copy(dsb0[D:D + 1, :],
                                              pso[0][D:D + 1, :])
                        nc.vector.tensor_copy(dsb1[D:D + 1, :],
                                              pso[1][D:D + 1, :])
                        nc.sync.dma_start(den[0:1, :], dsb0[D:D + 1, :])
                        nc.sync.dma_start(den[1:2, :], dsb1[D:D + 1, :])

                        def recip(den=den, lnd=lnd, rr2=rr2):
                            # 1/den = exp(-ln(den)): Ln+Exp share an ACT
                            # table set (DVE InstReciprocal costs ~4us).
                            # Deferred so the den DMA is long done and the
                            # ACT queue never stalls on it.
                            nc.scalar.activation(lnd[:], den[:], AF.Ln)
                            nc.scalar.activation(rr2[:], lnd[:], AF.Exp,
                                                 scale=-1.0)

                        def normalize(pair=pair, qs=qs, qt=qt, rr2=rr2):
                            psb = ps_bc.tile([P, 512], F32, tag="bc",
                                             name=f"psb_{pair}_{qt}")
                            nc.tensor.matmul(psb[:], lhsT=sel2[:],
                                             rhs=rr2[:], start=True,
                                             stop=True)
                            nc.vector.tensor_tensor(
                                ot_r[:, pair, qs], ot_u[:], psb[:], ALU.mult)
                        pending_recip.append(recip)
                        pending_norm.append(normalize)
                    while bg:
                        bg.pop(0)()
                while pending_recip:
                    pending_recip.pop(0)()
                while pending_norm:
                    pending_norm.pop(0)()
                for i in range(NPAIR):
                    for t in phase3_thunks((NQ - 1) * 4 + i):
                        t()

    _split_excess_waits(nc)
    return nc


def _get_nc():
    if "nc" not in _CACHE:
        _CACHE["nc"] = _build()
    return _CACHE["nc"]


def _bf16(a):
    return np.ascontiguousarray(np.asarray(a, np.float32)).astype(
        ml_dtypes.bfloat16)


def _sel2():
    s = np.zeros((2, P), dtype=np.float32)
    s[0, 0:D] = 1.0
    s[1, D:P] = 1.0
    return s


def kernel(x, Wq, bq, Wk, bk, Wv, bv, Wp, bp, **_unused):
    x = np.asarray(x, dtype=np.float32)
    bq = np.ascontiguousarray(np.asarray(bq, dtype=np.float32))
    bk = np.ascontiguousarray(np.asarray(bk, dtype=np.float32))
    bv = np.ascontiguousarray(np.asarray(bv, dtype=np.float32))
    bp = np.asarray(bp, dtype=np.float32)

    nc = _get_nc()
    in_maps = []
    for c in range(8):
        b, g = c // 2, c % 2
        js = slice(g * J, (g + 1) * J)
        in_maps.append({
            "xt": _bf16(np.asarray(x[b]).T),
            "wqt": _bf16(np.asarray(Wq)[js, :].T),
            "wkt": _bf16(np.asarray(Wk)[js, :].T),
            "wvt": _bf16(np.asarray(Wv)[js, :].T),
            "wpt": _bf16(np.asarray(Wp)[:, js].T),
            "bq": bq[js].copy(),
            "bk": bk[js].copy(),
            "bv": bv[js].copy(),
            "sel2": _sel2(),
        })
    res = run_bass_kernel_spmd(nc, in_maps, list(range(8)))
    out = np.empty((4, T, C), dtype=np.float32)
    for b in range(4):
        out[b] = res.results[2 * b]["y"] + res.results[2 * b + 1]["y"] + bp
    return out


# revision 35
# speedup vs baseline: 1.0584x; 1.0584x over previous
"""Distributed causal self-attention kernel for 8 TRN2 NeuronCores.

Problem (hardcoded): B=4, T=2048, C=1024, H=16 heads, D=64 head dim, fp32.
  y = softmax(causal(x Wq^T (x Wk^T)^T / sqrt(D))) (x Wv^T) Wp^T + biases

Sharding: data-parallel over B (4 groups) x tensor-parallel over heads
(2 groups of 8 heads).  Core c handles batch c//2, head-group c%2.  Each
core computes a partial output projection y_partial = O_g @ Wp[:,cols_g]^T;
the host sums the two partials of each batch pair (the 2-way all-reduce of
the sharding hint) and adds bp.

v3: one fused pipeline.  All matmul operands bf16 (host pre-casts; halves
DMA, 1 cycle/row on the PE).  The kernel is ACT(exp)-bound, so everything
else is arranged to hide under the exp stream and keep the PE busy enough
that the HAM clock gate stays at 2.4GHz:
  - QKV projection groups for t-blocks 1..3 are interleaved between the
    attention pair-rounds of earlier q-tiles (only block 0 runs up front);
    their PSUM accumulators share the broadcast/output-projection pool and
    their bias evictions run on the DVE, not ACT.
  - attention per (q-tile, head pair): S^T via two-head co-computation
    against zero-padded Q^T copies, exp on ACT (scale=1/8 fused, bf16 out),
    causal mask via gpsimd.affine_select, O^T accumulation with a ones
    column so PSUM row 64 is the softmax denominator.  O(kc-1) is emitted
    after S(kc) so the PE never sits behind exp(kc) in program order.
  - 1/denom = exp(-ln(denom)) on ACT (Ln+Exp share a table set; DVE
    InstReciprocal costs ~4us).  The denominator broadcast matmul and the
    final normalize multiply are DEFERRED into the next pair-round so the
    PSUM->SBUF->DMA->Ln->Exp chain latency hides behind real work.
  - the output projection y = O^T.T Wp^T for q-tile r is emitted one
    128-row tile per pair-round of q-tile r+1.
"""

import numpy as np
import ml_dtypes

import concourse.bass as bass
import concourse.mybir as mybir
from concourse.tile import TileContext
from concourse.bass_utils import run_bass_kernel_spmd

F32 = mybir.dt.float32
F32R = mybir.dt.float32r
BF16 = mybir.dt.bfloat16
AF = mybir.ActivationFunctionType
ALU = mybir.AluOpType

P = 128          # partitions
T = 2048         # sequence length
C = 1024         # model dim
D = 64           # head dim
HG = 8           # heads per core
J = HG * D       # per-core projection width (512)
CC = C // P      # contraction chunks over model dim (8)
JC = J // P      # j chunks (4)
NT = T // P      # 128-row t tiles (16)
TBS = 512        # t block size for phase-1 projections
NTB = T // TBS   # t blocks (4)
NQ = T // 512    # 512-wide query tiles (4)
NPAIR = HG // 2  # co-computed head pairs (4)

_CACHE = {}


def _split_excess_waits(nc):
    """Walrus in this container only accepts 1 sync-wait on CTRL-queue
    instructions (Drain etc.).  Hoist excess waits onto preceding nops on
    the same engine queue (program order makes this equivalent)."""
    n = 0
    for f in nc.m.functions:
        for bb in f.blocks:
            out = []
            for inst in bb.instructions:
                si = inst.sync_info
                limit = 1
                if si is not None and si.on_wait and len(si.on_wait) > limit:
                    waits = list(si.on_wait)
                    excess, keep = waits[:-limit], waits[-limit:]
                    for ci in range(0, len(excess), limit):
                        n += 1
                        out.append(mybir.InstNoOp(
                            name=f"waitsplit_{n}", opcode="nop", engine=inst.engine,
                            sync_info=mybir.SyncInfo(
                                on_wait=excess[ci:ci + limit], on_update=[]),
                        ))
                    inst.sync_info = mybir.SyncInfo(
                        on_wait=keep, on_update=list(si.on_update))
                out.append(inst)
            bb.instructions = out


def _build():
    nc = bass.Bass()
    # host passes x and the weight shards pre-transposed AND pre-cast to
    # bf16: xt = x[b].T, w?t = W?[rows].T, wpt = Wp[:,cols].T
    xt_in = nc.dram_tensor("xt", [C, T], BF16, kind="ExternalInput")
    wq_in = nc.dram_tensor("wqt", [C, J], BF16, kind="ExternalInput")
    wk_in = nc.dram_tensor("wkt", [C, J], BF16, kind="ExternalInput")
    wv_in = nc.dram_tensor("wvt", [C, J], BF16, kind="ExternalInput")
    wp_in = nc.dram_tensor("wpt", [J, C], BF16, kind="ExternalInput")
    bq_in = nc.dram_tensor("bq", [J], F32, kind="ExternalInput")
    bk_in = nc.dram_tensor("bk", [J], F32, kind="ExternalInput")
    bv_in = nc.dram_tensor("bv", [J], F32, kind="ExternalInput")
    sel2_in = nc.dram_tensor("sel2", [2, P], F32, kind="ExternalInput")
    y_out = nc.dram_tensor("y", [T, C], F32, kind="ExternalOutput")

    with TileContext(nc) as tc:
        with tc.tile_pool(name="persist", bufs=1) as persist:
            # persistent bf16 tensors
            qtz0 = persist.tile([P, JC, T], BF16, tag="qtz0")  # Q^T even heads
            qtz1 = persist.tile([P, JC, T], BF16, tag="qtz1")  # Q^T odd heads
            kt_t = persist.tile([P, JC, T], BF16, tag="kt")    # K^T
            v_t = persist.tile([P, NT, HG, D + 1], BF16, tag="v")  # V + ones
            ot_r = persist.tile([P, JC, T], BF16, tag="otr")   # normalized O^T
            wpt = persist.tile([P, JC, C], BF16, tag="wpt")    # Wp_g^T
            xt_all = persist.tile([P, CC, T], BF16, tag="xt")  # x^T
            wt_q = persist.tile([P, CC, J], BF16, tag="wtq")
            wt_k = persist.tile([P, CC, J], BF16, tag="wtk")
            wt_v = persist.tile([P, CC, J], BF16, tag="wtv")
            ones_row = persist.tile([1, P], F32R, tag="ones")
            bq_sb = persist.tile([P, JC], F32, tag="bq")
            bk_sb = persist.tile([P, JC], F32, tag="bk")
            bv_sb = persist.tile([1, J], F32, tag="bv")
            bv_r = persist.tile([1, J], F32R, tag="bvr")
            bv_bc = persist.tile([P, J], F32, tag="bvbc")
            ot_u = persist.tile([P, 512], BF16, tag="otu")
            tmp1 = persist.tile([D, 512], BF16, tag="tmp1")
            sel2 = persist.tile([2, P], F32R, tag="sel2")
            sel2f = persist.tile([2, P], F32, tag="sel2f")
            ones_f32 = persist.tile([P, P], F32, tag="ones_f32")
            ones_bf = persist.tile([P, P], BF16, tag="ones_bf")

            nc.gpsimd.memset(ones_f32[:], 1.0)
            nc.gpsimd.memset(ones_bf[:], 1.0)
            nc.vector.tensor_copy(ones_row[:], ones_f32[0:1, :])
            nc.gpsimd.memset(v_t[:, :, :, D:D + 1], 1.0)
            nc.gpsimd.memset(qtz0[D:P, :, :], 0.0)
            nc.gpsimd.memset(qtz1[0:D, :, :], 0.0)
            # small DMAs first (biases needed at the first evictions)
            nc.sync.dma_start(bq_sb[:], bq_in.rearrange("(o p) -> p o", p=P))
            nc.sync.dma_start(bk_sb[:], bk_in.rearrange("(o p) -> p o", p=P))
            nc.sync.dma_start(bv_sb[:], bv_in[None, :])
            nc.sync.dma_start(sel2f[:], sel2_in[:, :])
            nc.vector.tensor_copy(sel2[:], sel2f[:])
            # weight + x^T DMA queue: wq, x(tb0), wk, wv, x(tb1..3), wp
            for cc in range(CC):
                nc.sync.dma_start(wt_q[:, cc, :], wq_in[cc * P:(cc + 1) * P, :])
            for cc in range(CC):
                nc.sync.dma_start(xt_all[:, cc, 0:TBS],
                                  xt_in[cc * P:(cc + 1) * P, 0:TBS])
            for cc in range(CC):
                nc.sync.dma_start(wt_k[:, cc, :], wk_in[cc * P:(cc + 1) * P, :])
            for cc in range(CC):
                nc.sync.dma_start(wt_v[:, cc, :], wv_in[cc * P:(cc + 1) * P, :])
            for tb in range(1, NTB):
                ts = slice(tb * TBS, (tb + 1) * TBS)
                for cc in range(CC):
                    nc.sync.dma_start(xt_all[:, cc, ts],
                                      xt_in[cc * P:(cc + 1) * P, ts])
            for jc in range(JC):
                nc.sync.dma_start(wpt[:, jc, :], wp_in[jc * P:(jc + 1) * P, :])
            wts = {"q": wt_q, "k": wt_k, "v": wt_v}

            with (
                tc.tile_pool(name="e", bufs=4) as e_pool,
                tc.tile_pool(name="rc", bufs=2) as rc_pool,
                tc.tile_pool(name="yout", bufs=2) as y_pool,
                tc.tile_pool(name="ps_s", bufs=2, space="PSUM") as ps_s,
                tc.tile_pool(name="ps_o", bufs=2, space="PSUM") as ps_o,
                tc.tile_pool(name="ps_bc", bufs=2, space="PSUM") as ps_bc,
            ):
                # bv broadcast to all 128 partitions via K=1 matmul
                nc.vector.tensor_copy(bv_r[:], bv_sb[:])
                ps_bv = ps_bc.tile([P, 512], F32, tag="bc", name="ps_bv")
                nc.tensor.matmul(ps_bv[:], lhsT=ones_row[:], rhs=bv_r[:],
                                 start=True, stop=True)
                nc.vector.tensor_copy(bv_bc[:], ps_bv[:])

                # HAM warm-up: ~3.5us of full-array matmuls
                ps_warm = ps_s.tile([P, 1024], F32, tag="s", name="ps_warm")
                for _ in range(32):
                    nc.tensor.matmul(ps_warm[:, 0:P], lhsT=ones_bf[:],
                                     rhs=ones_bf[:], start=True, stop=True)

                # ---- QKV projection groups (bias evictions on DVE) --------
                # QKV groups are split into two ~0.85us micro-thunks (4
                # accumulating matmuls each) so they can be paced between
                # attention chunks without ever delaying the next S matmul
                # by more than the ACT slack.
                def qk_group(tb, name, jc):
                    ts = slice(tb * TBS, (tb + 1) * TBS)
                    wt, bias = wts[name], (bq_sb if name == "q" else bk_sb)
                    state = {}

                    def first():
                        psq = ps_bc.tile([P, 512], F32, tag="bc",
                                         name=f"psq_{tb}_{name}_{jc}")
                        state["psq"] = psq
                        for cc in range(4):
                            nc.tensor.matmul(
                                psq[:], lhsT=wt[:, cc, jc * P:(jc + 1) * P],
                                rhs=xt_all[:, cc, ts],
                                start=(cc == 0), stop=False)

                    def second():
                        psq = state["psq"]
                        for cc in range(4, CC):
                            nc.tensor.matmul(
                                psq[:], lhsT=wt[:, cc, jc * P:(jc + 1) * P],
                                rhs=xt_all[:, cc, ts],
                                start=False, stop=(cc == CC - 1))
                        if name == "q":
                            nc.vector.tensor_scalar_add(
                                qtz0[0:D, jc, ts], psq[0:D, :],
                                bias[0:D, jc:jc + 1])
                            nc.vector.tensor_scalar_add(
                                qtz1[D:P, jc, ts], psq[D:P, :],
                                bias[D:P, jc:jc + 1])
                        else:
                            nc.vector.tensor_scalar_add(
                                kt_t[:, jc, ts], psq[:], bias[:, jc:jc + 1])
                    return [first, second]

                def v_group(tb, sub):
                    tt = tb * (TBS // P) + sub
                    state = {}

                    def first():
                        psv = ps_bc.tile([P, 512], F32, tag="bc",
                                         name=f"psv_{tb}_{sub}")
                        state["psv"] = psv
                        for cc in range(4):
                            nc.tensor.matmul(
                                psv[:],
                                lhsT=xt_all[:, cc, tt * P:(tt + 1) * P],
                                rhs=wts["v"][:, cc, :],
                                start=(cc == 0), stop=False)

                    def second():
                        psv = state["psv"]
                        for cc in range(4, CC):
                            nc.tensor.matmul(
                                psv[:],
                                lhsT=xt_all[:, cc, tt * P:(tt + 1) * P],
                                rhs=wts["v"][:, cc, :],
                                start=False, stop=(cc == CC - 1))
                        nc.vector.tensor_tensor(
                            v_t[:, tt, :, 0:D],
                            psv.rearrange("p (h d) -> p h d", h=HG),
                            bv_bc.rearrange("p (h d) -> p h d", h=HG),
                            ALU.add)
                    return [first, second]

                def phase1_thunks(tb):
                    gs = []
                    for jc in range(JC):
                        gs.extend(qk_group(tb, "k", jc))
                    for jc in range(JC):
                        gs.extend(qk_group(tb, "q", jc))
                    for sub in range(TBS // P):
                        gs.extend(v_group(tb, sub))
                    return gs

                # t block 0 runs up front; blocks 1..3 are paced into the
                # attention rounds of q-tiles 0..2
                for g in phase1_thunks(0):
                    g()

                # ---- output projection, one 512-col half per thunk --------
                def phase3_thunks(tt):
                    ts = slice(tt * P, (tt + 1) * P)
                    state = {}

                    def half(nh):
                        if nh == 0:
                            state["y"] = y_pool.tile([P, C], F32, tag="y",
                                                     name=f"y_{tt}")
                        psy = ps_bc.tile([P, 512], F32, tag="bc",
                                         name=f"psy_{tt}_{nh}")
                        for jc in range(JC):
                            nc.tensor.matmul(
                                psy[:],
                                lhsT=ot_r[:, jc, ts],
                                rhs=wpt[:, jc, nh * 512:(nh + 1) * 512],
                                start=(jc == 0), stop=(jc == JC - 1))
                        nc.vector.tensor_copy(
                            state["y"][:, nh * 512:(nh + 1) * 512], psy[:])
                        if nh == 1:
                            nc.sync.dma_start(y_out[ts, :], state["y"][:])
                    return [lambda: half(0), lambda: half(1)]

                # ---- attention ------------------------------------------
                pending_recip = []
                pending_norm = []

                for qt in range(NQ):
                    nk = (qt + 1) * 4
                    qs = slice(qt * 512, (qt + 1) * 512)
                    # background PE work paced into this q-tile's rounds:
                    # next t-block's projections + previous q-tile's output
                    # projection, ~0.85us per thunk, one per chunk slot
                    # the previous q-tile's last normalize is still deferred;
                    # it must be emitted before any phase3 thunk that reads
                    # ot_r (emission order defines dependencies)
                    while pending_recip:
                        pending_recip.pop(0)()
                    while pending_norm:
                        pending_norm.pop(0)()
                    # background PE work bunched at round ends (frequent
                    # small stalls cost more than few big ones): next
                    # t-block's projections + previous q-tile's y tiles
                    bg = []
                    if qt < NQ - 1:
                        bg.extend(phase1_thunks(qt + 1))
                    if qt > 0:
                        for i in range(4):
                            bg.extend(phase3_thunks((qt - 1) * 4 + i))
                    per_round = -(-len(bg) // NPAIR) if bg else 0
                    for pair in range(NPAIR):
                        pso = [ps_o.tile([P, 512], F32, tag="o",
                                         name=f"pso_{pair}_{qt}_{i}")
                               for i in range(2)]

                        def emit_o(kc, e, delta, pair=pair, nk=nk, pso=pso):
                            for half in range(2):
                                h = pair * 2 + half
                                nc.tensor.matmul(
                                    pso[half][0:D + 1, delta:],
                                    lhsT=v_t[:, kc, h, :],
                                    rhs=e[:, half * 512 + delta:
                                          (half + 1) * 512],
                                    start=(kc == 0), stop=(kc == nk - 1))

                        # software-pipelined: O(kc-1) is emitted after S(kc)
                        # so the PE never waits on exp(kc) in program order
                        prev = None
                        for kc in range(nk):
                            ks = slice(kc * P, (kc + 1) * P)
                            delta = max(0, (kc - qt * 4) * P)
                            # first two diagonal chunks: compute S full
                            # width (cheaper than splitting the exp)
                            sd = delta if delta >= 256 else 0
                            pss = ps_s.tile([P, 1024], F32, tag="s")
                            for half in range(2):
                                qtz = qtz0 if half == 0 else qtz1
                                nc.tensor.matmul(
                                    pss[:, half * 512 + sd:(half + 1) * 512],
                                    lhsT=kt_t[:, pair, ks],
                                    rhs=qtz[:, pair,
                                            qt * 512 + sd:(qt + 1) * 512],
                                    start=True, stop=True)
                            if prev is not None:
                                emit_o(*prev)
                            if kc == 1 and pending_recip:
                                pending_recip.pop(0)()
                            if kc == 3 and pending_norm:
                                pending_norm.pop(0)()
                            e = e_pool.tile([P, 1024], BF16, tag="e")
                            if sd == 0:
                                nc.scalar.activation(
                                    e[:], pss[:], AF.Exp, scale=0.125)
                            else:
                                for half in range(2):
                                    sl = slice(half * 512 + sd,
                                               (half + 1) * 512)
                                    nc.scalar.activation(
                                        e[:, sl], pss[:, sl], AF.Exp,
                                        scale=0.125)
                            if kc >= qt * 4:  # diagonal: causal mask
                                nc.gpsimd.affine_select(
                                    out=e[:], in_=e[:],
                                    compare_op=ALU.is_ge, fill=0.0,
                                    base=-delta, channel_multiplier=-1,
                                    pattern=[[0, 2], [1, 512]])
                            prev = (kc, e, delta)
                        emit_o(*prev)

                        # eviction: O rows to SBUF (odd half crosses
                        # partitions via DMA), denominators to partitions
                        # 0/1, 1/den = exp(-ln(den)) on ACT.
                        nc.vector.tensor_copy(ot_u[0:D, :], pso[0][0:D, :])
                        nc.vector.tensor_copy(tmp1[:], pso[1][0:D, :])
                        nc.sync.dma_start(ot_u[D:P, :], tmp1[:])
                        den = rc_pool.tile([2, 512], F32, tag="den",
                                           name=f"den_{pair}_{qt}")
                        lnd = rc_pool.tile([2, 512], F32, tag="lnd",
                                           name=f"lnd_{pair}_{qt}")
                        rr2 = rc_pool.tile([2, 512], F32R, tag="rr2",
                                           name=f"rr_{pair}_{qt}")
                        dsb0 = rc_pool.tile([D + 1, 512], F32, tag="dsb0",
                                            name=f"dsb0_{pair}_{qt}")
                        dsb1 = rc_pool.tile([D + 1, 512], F32, tag="dsb1",
                                            name=f"dsb1_{pair}_{qt}")
                        nc.vector.tensor_copy(dsb0[D:D + 1, :],
                                              pso[0][D:D + 1, :])
                        nc.vector.tensor_copy(dsb1[D:D + 1, :],
                                              pso[1][D:D + 1, :])
                        nc.sync.dma_start(den[0:1, :], dsb0[D:D + 1, :])
                        nc.sync.dma_start(den[1:2, :], dsb1[D:D + 1, :])

                        def recip(den=den, lnd=lnd, rr2=rr2):
                            # 1/den = exp(-ln(den)): Ln+Exp share an ACT
                            # table set (DVE InstReciprocal costs ~4us).
                            # Deferred so the den DMA is long done and the
                            # ACT queue never stalls on it.
                            nc.scalar.activation(lnd[:], den[:], AF.Ln)
                            nc.scalar.activation(rr2[:], lnd[:], AF.Exp,
                                                 scale=-1.0)

                        def normalize(pair=pair, qs=qs, qt=qt, rr2=rr2):
                            psb = ps_bc.tile([P, 512], F32, tag="bc",
                                             name=f"psb_{pair}_{qt}")
                            nc.tensor.matmul(psb[:], lhsT=sel2[:],
                                             rhs=rr2[:], start=True,
                                             stop=True)
                            nc.vector.tensor_tensor(
                                ot_r[:, pair, qs], ot_u[:], psb[:], ALU.mult)
                        pending_recip.append(recip)
                        pending_norm.append(normalize)
                        for _ in range(min(per_round, len(bg))):
                            bg.pop(0)()
                    while bg:
                        bg.pop(0)()
                while pending_recip:
                    pending_recip.pop(0)()
                while pending_norm:
                    pending_norm.pop(0)()
                for i in range(NPAIR):
                    for t in phase3_thunks((NQ - 1) * 4 + i):
                        t()

    _split_excess_waits(nc)
    return nc


def _get_nc():
    if "nc" not in _CACHE:
        _CACHE["nc"] = _build()
    return _CACHE["nc"]


def _bf16(a):
    return np.ascontiguousarray(np.asarray(a, np.float32)).astype(
        ml_dtypes.bfloat16)


def _sel2():
    s = np.zeros((2, P), dtype=np.float32)
    s[0, 0:D] = 1.0
    s[1, D:P] = 1.0
    return s


def kernel(x, Wq, bq, Wk, bk, Wv, bv, Wp, bp, **_unused):
    x = np.asarray(x, dtype=np.float32)
    bq = np.ascontiguousarray(np.asarray(bq, dtype=np.float32))
    bk = np.ascontiguousarray(np.asarray(bk, dtype=np.float32))
    bv = np.ascontiguousarray(np.asarray(bv, dtype=np.float32))
    bp = np.asarray(bp, dtype=np.float32)

    nc = _get_nc()
    in_maps = []
    for c in range(8):
        b, g = c // 2, c % 2
        js = slice(g * J, (g + 1) * J)
        in_maps.append({
            "xt": _bf16(np.asarray(x[b]).T),
            "wqt": _bf16(np.asarray(Wq)[js, :].T),
            "wkt": _bf16(np.asarray(Wk)[js, :].T),
            "wvt": _bf16(np.asarray(Wv)[js, :].T),
            "wpt": _bf16(np.asarray(Wp)[:, js].T),
            "bq": bq[js].copy(),
            "bk": bk[js].copy(),
            "bv": bv[js].copy(),
            "sel2": _sel2(),
        })
    res = run_bass_kernel_spmd(nc, in_maps, list(range(8)))
    out = np.empty((4, T, C), dtype=np.float32)
    for b in range(4):
        out[b] = res.results[2 * b]["y"] + res.results[2 * b + 1]["y"] + bp
    return out
